# revision 15
# baseline (speedup 1.0000x reference)
"""Trainium2 Bass kernel for nn_CapsuleNet (gnn_message_passing).

Strategy (sharding_hint): shard nodes across the 8 NeuronCores (contiguous
blocks).  Each layer, every core normalizes its own nodes' features, the
per-core tables are AllGathered into a full bf16 node-feature table in DRAM,
and each core dma_gathers its local nodes' neighbor rows (z) once per layer.
The 6 dynamic-routing iterations, the argmax one-hot and the meta-path
aggregation are then fully node-local.

Layout: nodes on partitions, packed 4 per partition (tile = 512 nodes);
feature vectors stored (c,k)-permuted so the capsule axis k is innermost
(makes the p-broadcast multiply a unit-stride bf16 2x op).  The two big
routing einsums run as bf16 tensor_tensor multiplies + halving-tree adds.
rsqrt/normalize uses exp(-0.5*ln(x)) so the whole kernel stays on one ACT
table set (natural_log_exp_and_others).
"""
import os
import sys

sys.path.insert(0, '/opt/trn_rl_repo')

STAGE = int(os.environ.get("KSTAGE", "4"))
RSUB = int(os.environ.get("KRSUB", "9"))

import numpy as np
import ml_dtypes

import concourse.bass as bass
import concourse.bacc as bacc
import concourse.mybir as mybir
from concourse.tile import TileContext
from concourse.bass_utils import run_bass_kernel_spmd

dt = mybir.dt
AX = mybir.AxisListType
ALU = mybir.AluOpType
ACTF = mybir.ActivationFunctionType

# --- problem constants (hardcoded per spec) ---
K, C, D = 8, 16, 128          # capsules, hidden-per-capsule, d = K*C
M = 32                        # neighbors per node
F, FP = 500, 512              # input features (padded to 4x128)
NCLASS = 16
ROUTIT = 6
CUT = 5
NCORES = 8
N_TOTAL = 20000
NLOC = N_TOTAL // NCORES      # 2500 nodes per core
P = 128                       # partitions
N4 = 2                        # nodes per partition per tile
TN = P * N4                   # 512 nodes per tile
BF = dt.bfloat16
F32 = dt.float32

_CACHE = {}


class _StopTrace(Exception):
    pass


def _cnt(nloc, t, n4):
    """valid node count for (tile, n4) block"""
    return max(0, min(128, nloc - (t * TN + n4 * P)))


def build_program(ncores, nloc):
    NT = (nloc + TN - 1) // TN
    n_total = ncores * nloc
    NZ = TN * M               # z-gather indices per tile
    NG = TN * CUT             # meta-gather indices per tile

    nc = bacc.Bacc("TRN2", target_bir_lowering=False, debug=False,
                   num_devices=ncores)

    # ---- I/O ----
    x_in = nc.dram_tensor("x", [nloc, F], F32, kind="ExternalInput")
    idxz_in = nc.dram_tensor("idxz", [NT, P, NZ // 16], dt.int16, kind="ExternalInput")
    idxg_in = nc.dram_tensor("idxg", [NT, P, NG // 16], dt.int16, kind="ExternalInput")
    pcaw_in = nc.dram_tensor("pcaw", [F, D], F32, kind="ExternalInput")
    pcab_in = nc.dram_tensor("pcab", [D], F32, kind="ExternalInput")
    wqsT_in = nc.dram_tensor("wqsT", [C, C], BF, kind="ExternalInput")
    wksT_in = nc.dram_tensor("wksT", [C, C], BF, kind="ExternalInput")
    lng_in = nc.dram_tensor("lng", [C], F32, kind="ExternalInput")
    lnb_in = nc.dram_tensor("lnb", [C], F32, kind="ExternalInput")
    mlpw_in = nc.dram_tensor("mlpw", [D, NCLASS], F32, kind="ExternalInput")  # (c,k)-permuted rows
    mlpb_in = nc.dram_tensor("mlpb", [NCLASS], F32, kind="ExternalInput")
    ident_in = nc.dram_tensor("ident", [P, P], F32, kind="ExternalInput")
    tbv_in = nc.dram_tensor("tbv", [K], F32, kind="ExternalInput")

    out_lsm = nc.dram_tensor("out_lsm", [nloc, NCLASS], F32, kind="ExternalOutput")
    out_meta = nc.dram_tensor("out_meta", [nloc, NCLASS], F32, kind="ExternalOutput")
    out_attn = nc.dram_tensor("out_attn", [nloc, 1], F32, kind="ExternalOutput")

    with TileContext(nc) as tc:
      try:
        with (
            tc.tile_pool(name="consts", bufs=1) as cp,
            tc.tile_pool(name="res", bufs=1) as rp,
            tc.tile_pool(name="zp", bufs=2) as zpool,
            tc.tile_pool(name="zu", bufs=1) as zupool,
            tc.tile_pool(name="wk2", bufs=2) as wk2,
            tc.tile_pool(name="wk", bufs=1) as wk,
            tc.tile_pool(name="sm", bufs=1) as sm,
            tc.tile_pool(name="psum", bufs=2, space="PSUM") as ps,
            tc.tile_pool(name="dram", bufs=1, space="DRAM") as dram,
        ):
            # ---------------- constants ----------------
            ident = cp.tile([P, P], F32)
            nc.sync.dma_start(out=ident[:], in_=ident_in[:, :])
            wt = cp.tile([P, 4, D], F32)          # pca_w f-chunks
            nc.gpsimd.memset(wt[:], 0.0)
            nc.sync.dma_start(out=wt[:, 0:3, :],
                              in_=pcaw_in[0:384, :].rearrange("(c p) d -> p c d", p=P))
            nc.sync.dma_start(out=wt[0:F - 384, 3, :], in_=pcaw_in[384:F, :])
            bt = cp.tile([P, D], F32)
            nc.sync.dma_start(out=bt[:], in_=pcab_in[None, :].to_broadcast([P, D]))
            wqsT = cp.tile([P, C, C], BF)
            nc.sync.dma_start(out=wqsT[:], in_=wqsT_in[None, :, :].to_broadcast([P, C, C]))
            wksT = cp.tile([P, C, C], BF)
            nc.sync.dma_start(out=wksT[:], in_=wksT_in[None, :, :].to_broadcast([P, C, C]))
            lng = cp.tile([P, C], F32)
            nc.sync.dma_start(out=lng[:], in_=lng_in[None, :].to_broadcast([P, C]))
            lnb = cp.tile([P, C], F32)
            nc.sync.dma_start(out=lnb[:], in_=lnb_in[None, :].to_broadcast([P, C]))
            mlpw = cp.tile([P, NCLASS], F32)
            nc.sync.dma_start(out=mlpw[:], in_=mlpw_in[:, :])
            eps6 = cp.tile([P, 1], F32)
            nc.gpsimd.memset(eps6[:], 1e-6)
            kidx = cp.tile([P, K], F32)
            nc.sync.dma_start(out=kidx[:], in_=tbv_in[None, :].to_broadcast([P, K]))
            kplus = cp.tile([P, K], F32)
            nc.vector.tensor_scalar_add(kplus[:], kidx[:], 999.0)
            mlpb = cp.tile([P, NCLASS], F32)
            nc.sync.dma_start(out=mlpb[:], in_=mlpb_in[None, :].to_broadcast([P, NCLASS]))
            idxz = []
            idxg = []
            for t in range(NT):
                iz = cp.tile([P, NZ // 16], dt.int16, tag=f"idxz{t}", name=f"idxz{t}")
                nc.sync.dma_start(out=iz[:], in_=idxz_in[t, :, :])
                idxz.append(iz)
                ig = cp.tile([P, NG // 16], dt.int16, tag=f"idxg{t}")
                nc.sync.dma_start(out=ig[:], in_=idxg_in[t, :, :])
                idxg.append(ig)

            # ---------------- DRAM tables ----------------
            loc_xc = dram.tile([nloc, D], F32)    # this core's x_c rows (per layer)
            tab = dram.tile([n_total, D], F32)    # gathered full table (per layer)
            loc_tj = dram.tile([nloc, D], BF)     # padded Tj~ rows
            tab_tj = dram.tile([n_total, D], BF)

            # ---------------- residents ----------------
            x3a = [rp.tile([P, N4, C, K], F32, tag=f"x3a{t}", name=f"x3a{t}") for t in range(NT)]
            x3b = [rp.tile([P, N4, C, K], F32, tag=f"x3b{t}", name=f"x3b{t}") for t in range(NT)]
            uo = [rp.tile([P, N4, C, K], F32, tag=f"uo{t}", name=f"uo{t}") for t in range(NT)]
            ohs = [rp.tile([P, N4, CUT, K], BF, tag=f"oh{t}", name=f"oh{t}") for t in range(NT)]

            # ===================================================
            # helpers
            # ===================================================
            def normalize(u, rn_out_tag):
                """rn = 1/max(||u||_c, 1e-12) per (n4,k); returns rn [P,N4,K] f32"""
                usq = sm.tile([P, N4, C, K], F32, tag="usq")
                nc.scalar.activation(usq[:], u[:], ACTF.Square)
                ss = sm.tile([P, N4, K], F32, tag=rn_out_tag + "ss")
                nc.vector.tensor_reduce(
                    ss[:], usq.rearrange("p n c k -> p n k c"),
                    axis=AX.X, op=ALU.add)
                nc.vector.tensor_scalar_max(ss[:], ss[:], 1e-24)
                lg = sm.tile([P, N4, K], F32, tag=rn_out_tag + "lg")
                nc.scalar.activation(lg[:], ss[:], ACTF.Ln)
                rn = sm.tile([P, N4, K], F32, tag=rn_out_tag)
                nc.scalar.activation(rn[:], lg[:], ACTF.Exp, scale=-0.5)
                return rn

            # ===================================================
            # Phase A: pca + attention + x3a + x_c table rows
            # ===================================================
            for t in range(NT):
                xt = wk2.tile([P, N4, FP], F32, tag="xt")
                if _cnt(nloc, t, N4 - 1) < P:
                    nc.gpsimd.memset(xt[:], 0.0)
                else:
                    nc.gpsimd.memset(xt[:, :, F:FP], 0.0)
                for n4 in range(N4):
                    cnt = _cnt(nloc, t, n4)
                    if cnt > 0:
                        r0 = t * TN + n4 * P
                        nc.sync.dma_start(out=xt[0:cnt, n4, 0:F],
                                          in_=x_in[r0:r0 + cnt, :])
                # pca matmul -> h [p, n4, (k,c)]
                h = wk.tile([P, N4, K, C], F32, tag="h")
                for n4 in range(N4):
                    xTs = wk.tile([P, 4, P], F32, tag="xTs")
                    for ci in range(4):
                        tp = ps.tile([P, P], F32, tag="tp")
                        nc.tensor.transpose(
                            tp[:], xt[:, n4, ci * P:(ci + 1) * P], ident[:])
                        nc.vector.tensor_copy(xTs[:, ci, :], tp[:])
                    hp = ps.tile([P, D], F32, tag="hp")
                    for ci in range(4):
                        nc.tensor.matmul(hp[:], lhsT=xTs[:, ci, :], rhs=wt[:, ci, :],
                                         start=(ci == 0), stop=(ci == 3))
                    nc.vector.tensor_tensor(
                        h.rearrange("p n k c -> p n (k c)")[:, n4, :],
                        hp[:], bt[:], op=ALU.add)

                # ---- independence attention (pre-relu h3) ----
                mu = sm.tile([P, N4, K], F32, tag="mu")
                nc.vector.tensor_reduce(mu[:], h[:], axis=AX.X, op=ALU.add)
                nc.vector.tensor_scalar_mul(mu[:], mu[:], 1.0 / C)
                hm = wk.tile([P, N4, K, C], F32, tag="hm")
                nc.vector.tensor_tensor(
                    hm[:], h[:], mu[:, :, :, None].broadcast_to([P, N4, K, C]),
                    op=ALU.subtract)
                hsq = wk.tile([P, N4, K, C], F32, tag="usq2")
                nc.scalar.activation(hsq[:], hm[:], ACTF.Square)
                ssq = sm.tile([P, N4, K], F32, tag="ssq")
                nc.vector.tensor_reduce(ssq[:], hsq[:], axis=AX.X, op=ALU.add)
                # rstd = exp(-0.5*ln(ssq/16 + 1e-6))
                lgv = sm.tile([P, N4, K], F32, tag="lgv")
                nc.scalar.activation(lgv[:], ssq[:], ACTF.Ln, scale=1.0 / C, bias=eps6[:])
                rstd = sm.tile([P, N4, K], F32, tag="rstd")
                nc.scalar.activation(rstd[:], lgv[:], ACTF.Exp, scale=-0.5)
                ln = wk.tile([P, N4, K, C], F32, tag="ln")
                nc.vector.tensor_tensor(
                    ln[:], hm[:], rstd[:, :, :, None].broadcast_to([P, N4, K, C]),
                    op=ALU.mult)
                nc.vector.tensor_tensor(
                    ln[:], ln[:],
                    lng[:, None, None, :].broadcast_to([P, N4, K, C]), op=ALU.mult)
                nc.vector.tensor_tensor(
                    ln[:], ln[:],
                    lnb[:, None, None, :].broadcast_to([P, N4, K, C]), op=ALU.add)
                lnb16 = wk.tile([P, N4, K, C], BF, tag="lnb16")
                nc.vector.tensor_copy(lnb16[:], ln[:])
                hb16 = wk.tile([P, N4, K, C], BF, tag="hb16")
                nc.vector.tensor_copy(hb16[:], h[:])

                # q = LN @ wqs, kk = h3 @ wks  (per-capsule shared 16x16)
                prod = zupool.tile([P, N4, K, C, C], BF, tag="zu")
                q = wk.tile([P, N4, K, C], BF, tag="q")
                kk = wk.tile([P, N4, K, C], BF, tag="kk")
                for (src, wT, dst) in ((lnb16, wqsT, q), (hb16, wksT, kk)):
                    nc.vector.tensor_tensor(
                        prod[:],
                        src[:, :, :, None, :].broadcast_to([P, N4, K, C, C]),
                        wT[:, None, None, :, :].broadcast_to([P, N4, K, C, C]),
                        op=ALU.mult)
                    cs = C // 2
                    while cs >= 2:
                        nc.vector.tensor_tensor(
                            prod[:, :, :, :, 0:cs], prod[:, :, :, :, 0:cs],
                            prod[:, :, :, :, cs:2 * cs], op=ALU.add)
                        cs //= 2
                    nc.vector.tensor_tensor(
                        dst[:], prod[:, :, :, :, 0], prod[:, :, :, :, 1], op=ALU.add)
                # logits l[kq, kk'] = sum_c q[kq,c]*kk[kk',c]
                prodl = zupool.tile([P, N4, K, K, C], BF, tag="zu")
                nc.vector.tensor_tensor(
                    prodl[:],
                    q[:, :, :, None, :].broadcast_to([P, N4, K, K, C]),
                    kk[:, :, None, :, :].broadcast_to([P, N4, K, K, C]),
                    op=ALU.mult)
                cs = C // 2
                while cs >= 2:
                    nc.vector.tensor_tensor(
                        prodl[:, :, :, :, 0:cs], prodl[:, :, :, :, 0:cs],
                        prodl[:, :, :, :, cs:2 * cs], op=ALU.add)
                    cs //= 2
                lgt = wk.tile([P, N4, K, K], F32, tag="lgt")
                nc.vector.tensor_tensor(
                    lgt[:], prodl[:, :, :, :, 0], prodl[:, :, :, :, 1], op=ALU.add)
                pel = wk.tile([P, N4, K, K], F32, tag="pel")
                nc.scalar.activation(pel[:], lgt[:], ACTF.Exp, scale=0.25)  # /temp=4
                den = sm.tile([P, N4, K], F32, tag="den")
                nc.vector.tensor_reduce(den[:], pel[:], axis=AX.X, op=ALU.add)
                rden = sm.tile([P, N4, K], F32, tag="rden")
                nc.vector.reciprocal(rden[:], den[:])
                dg = sm.tile([P, N4, K], F32, tag="dg")
                nc.vector.tensor_tensor(
                    dg[:],
                    pel.rearrange("p n a b -> p n (a b)")[:, :, 0:K * K:K + 1],
                    rden[:], op=ALU.mult)
                sdg = sm.tile([P, N4], F32, tag="sdg")
                nc.vector.tensor_reduce(sdg[:], dg[:], axis=AX.X, op=ALU.add)
                att = sm.tile([P, N4], F32, tag="att")
                nc.vector.tensor_scalar(att[:], sdg[:], -1.0, float(K),
                                        op0=ALU.mult, op1=ALU.add)
                for n4 in range(N4):
                    cnt = _cnt(nloc, t, n4)
                    if cnt > 0:
                        r0 = t * TN + n4 * P
                        nc.sync.dma_start(out=out_attn[r0:r0 + cnt, :],
                                          in_=att[0:cnt, n4, None])

                # ---- x0 = relu(h); x3a = normalize(x0) stored (c,k) ----
                x0 = wk.tile([P, N4, K, C], F32, tag="x0")
                nc.vector.tensor_scalar_max(x0[:], h[:], 0.0)
                x0sq = wk.tile([P, N4, K, C], F32, tag="usq2")
                nc.scalar.activation(x0sq[:], x0[:], ACTF.Square)
                ss0 = sm.tile([P, N4, K], F32, tag="ss0")
                nc.vector.tensor_reduce(ss0[:], x0sq[:], axis=AX.X, op=ALU.add)
                nc.vector.tensor_scalar_max(ss0[:], ss0[:], 1e-24)
                lg0 = sm.tile([P, N4, K], F32, tag="lg0")
                nc.scalar.activation(lg0[:], ss0[:], ACTF.Ln)
                rn0 = sm.tile([P, N4, K], F32, tag="rn0")
                nc.scalar.activation(rn0[:], lg0[:], ACTF.Exp, scale=-0.5)
                # permuted write: x3a[(c,k)] = x0[(k,c)] * rn0[k]
                nc.vector.tensor_tensor(
                    x3a[t].rearrange("p n c k -> p n k c"),
                    x0[:], rn0[:, :, :, None].broadcast_to([P, N4, K, C]),
                    op=ALU.mult)
                for n4 in range(N4):
                    cnt = _cnt(nloc, t, n4)
                    if cnt > 0:
                        r0 = t * TN + n4 * P
                        nc.sync.dma_start(
                            out=loc_xc[r0:r0 + cnt, :],
                            in_=x3a[t].rearrange("p n c k -> p n (c k)")[0:cnt, n4, :])

            # ===================================================
            # routing layer runner
            # ===================================================
            def routing_layer(layer, x3, tab_ap):
                """returns final u per tile (f32 (c,k)); last layer also p_raw5+z kept"""
                for t in range(NT):
                    if RSUB == 0:
                        continue
                    z = zpool.tile([P, N4, M, D], F32, tag="z")
                    nc.gpsimd.dma_gather(
                        out_ap=z.rearrange("p n m d -> p (n m) d"),
                        in_ap=tab_ap[:],
                        idxs_ap=idxz[t][:],
                        num_idxs=NZ, num_idxs_reg=NZ, elem_size=D,
                        single_packet=False)
                    z5 = z.rearrange("p n m (c k) -> p n m c k", c=C)
                    if RSUB == 1:
                        continue

                    u = sm.tile([P, N4, C, K], F32, tag="u")
                    zu = zupool.tile([P, N4, M, D], F32, tag="zu")
                    zu5 = zu.rearrange("p n m (c k) -> p n m c k", c=C)

                    # it0: u = (1/8) sum_m z + x3
                    usum = sm.tile([P, N4, C, K], F32, tag="usum")
                    nc.vector.tensor_reduce(
                        usum.rearrange("p n c k -> p n (c k)"),
                        z.rearrange("p n m d -> p n d m"),
                        axis=AX.X, op=ALU.add)
                    nc.vector.scalar_tensor_tensor(
                        u[:], usum[:], 1.0 / K, x3[t][:],
                        op0=ALU.mult, op1=ALU.add)
                    rn = normalize(u, "rnit")
                    un = sm.tile([P, N4, C, K], F32, tag="un")
                    nc.vector.tensor_tensor(
                        un[:], u[:], rn[:, :, None, :].broadcast_to([P, N4, C, K]),
                        op=ALU.mult)

                    praw = sm.tile([P, N4, M, K], F32, tag="praw")
                    if RSUB == 2:
                        continue
                    for it in range(1, min(ROUTIT, RSUB)):
                        last_it = it == ROUTIT - 1
                        # p = softmax_k(z . u)
                        nc.vector.tensor_tensor(
                            zu[:], z[:],
                            un.rearrange("p n c k -> p n (c k)")[:, :, None, :]
                              .broadcast_to([P, N4, M, D]),
                            op=ALU.mult)
                        nc.vector.tensor_reduce(
                            praw[:], zu5.rearrange("p n m c k -> p n m k c"),
                            axis=AX.X, op=ALU.add)
                        pe = sm.tile([P, N4, M, K], F32, tag="pe")
                        nc.scalar.activation(pe[:], praw[:], ACTF.Exp)
                        den = sm.tile([P, N4, M], F32, tag="denr")
                        nc.vector.tensor_reduce(den[:], pe[:], axis=AX.X, op=ALU.add)
                        rec = sm.tile([P, N4, M], F32, tag="rec")
                        nc.vector.reciprocal_approx_fast(rec[:], den[:])
                        pn = sm.tile([P, N4, M, K], F32, tag="pn")
                        nc.vector.tensor_tensor(
                            pn[:], pe[:],
                            rec[:, :, :, None].broadcast_to([P, N4, M, K]),
                            op=ALU.mult)
                        # u = sum_m pn * z + x3
                        nc.vector.tensor_tensor(
                            zu5[:], z5[:],
                            pn[:, :, :, None, :].broadcast_to([P, N4, M, C, K]),
                            op=ALU.mult)
                        nc.vector.tensor_reduce(
                            u[:], zu5.rearrange("p n m c k -> p n c k m"),
                            axis=AX.X, op=ALU.add)
                        nc.vector.tensor_tensor(u[:], u[:], x3[t][:], op=ALU.add)
                        if not last_it:
                            rn = normalize(u, "rnit")
                            nc.vector.tensor_tensor(
                                un[:], u[:],
                                rn[:, :, None, :].broadcast_to([P, N4, C, K]),
                                op=ALU.mult)

                    if layer == 0:
                        # h1 = relu(u); x3b = normalize(h1); write x_c rows
                        x1 = sm.tile([P, N4, C, K], F32, tag="x1")
                        nc.vector.tensor_scalar_max(x1[:], u[:], 0.0)
                        x1sq = sm.tile([P, N4, C, K], F32, tag="usq")
                        nc.scalar.activation(x1sq[:], x1[:], ACTF.Square)
                        ss1 = sm.tile([P, N4, K], F32, tag="ss1")
                        nc.vector.tensor_reduce(
                            ss1[:], x1sq.rearrange("p n c k -> p n k c"),
                            axis=AX.X, op=ALU.add)
                        nc.vector.tensor_scalar_max(ss1[:], ss1[:], 1e-24)
                        lg1 = sm.tile([P, N4, K], F32, tag="lg1")
                        nc.scalar.activation(lg1[:], ss1[:], ACTF.Ln)
                        rn1 = sm.tile([P, N4, K], F32, tag="rn1")
                        nc.scalar.activation(rn1[:], lg1[:], ACTF.Exp, scale=-0.5)
                        nc.vector.tensor_tensor(
                            x3b[t][:], x1[:],
                            rn1[:, :, None, :].broadcast_to([P, N4, C, K]),
                            op=ALU.mult)
                        for n4 in range(N4):
                            cnt = _cnt(nloc, t, n4)
                            if cnt > 0:
                                r0 = t * TN + n4 * P
                                nc.sync.dma_start(
                                    out=loc_xc[r0:r0 + cnt, :],
                                    in_=x3b[t].rearrange("p n c k -> p n (c k)")[0:cnt, n4, :])
                    else:
                        # keep u; one-hot argmax of first CUT slots; Tj~ rows
                        nc.vector.tensor_copy(uo[t][:], u[:])
                        p5 = praw[:, :, 0:CUT, :]
                        mx = sm.tile([P, N4, CUT], F32, tag="mx5")
                        nc.vector.tensor_reduce(mx[:], p5, axis=AX.X, op=ALU.max)
                        ohm = sm.tile([P, N4, CUT, K], F32, tag="ohm")
                        nc.vector.tensor_tensor(
                            ohm[:], p5,
                            mx[:, :, :, None].broadcast_to([P, N4, CUT, K]),
                            op=ALU.is_ge)
                        # exact first-max: min k among maxima
                        cand = sm.tile([P, N4, CUT, K], F32, tag="cand")
                        nc.vector.scalar_tensor_tensor(
                            cand.rearrange("p n a k -> p (n a) k"),
                            ohm.rearrange("p n a k -> p (n a) k"), -999.0,
                            kplus[:, None, :].broadcast_to([P, N4 * CUT, K]),
                            op0=ALU.mult, op1=ALU.add)
                        mnk = sm.tile([P, N4, CUT], F32, tag="mnk")
                        nc.vector.tensor_reduce(mnk[:], cand[:], axis=AX.X, op=ALU.min)
                        nc.vector.tensor_tensor(
                            ohs[t][:],
                            kidx[:, None, None, :].broadcast_to([P, N4, CUT, K]),
                            mnk[:, :, :, None].broadcast_to([P, N4, CUT, K]),
                            op=ALU.is_equal)
                        # Tj~[c] = sum_{a<CUT} sum_k oh[a,k] * z[a,c,k]
                        zz = zupool.tile([P, N4, CUT, C, K], BF, tag="zu")
                        nc.vector.tensor_tensor(
                            zz[:], z5[:, :, 0:CUT, :, :],
                            ohs[t][:, :, :, None, :].broadcast_to([P, N4, CUT, C, K]),
                            op=ALU.mult)
                        tj = sm.tile([P, N4, C], F32, tag="tj")
                        nc.vector.tensor_reduce(
                            tj[:], zz.rearrange("p n a c k -> p n c a k"),
                            axis=AX.XY, op=ALU.add)
                        tjb = sm.tile([P, N4, C], BF, tag="tjb")
                        nc.vector.tensor_copy(tjb[:], tj[:])
                        for n4 in range(N4):
                            cnt = _cnt(nloc, t, n4)
                            if cnt > 0:
                                r0 = t * TN + n4 * P
                                nc.sync.dma_start(out=loc_tj[r0:r0 + cnt, 0:C],
                                                  in_=tjb[0:cnt, n4, :])

            # ---- layer 0 ----
            if STAGE < 2:
                raise _StopTrace()
            nc.gpsimd.collective_compute(
                "AllGather", ALU.bypass,
                replica_groups=[list(range(ncores))],
                ins=[loc_xc.opt()], outs=[tab.opt()])
            routing_layer(0, x3a, tab)

            # ---- layer 1 ----
            if STAGE < 3:
                raise _StopTrace()
            nc.gpsimd.collective_compute(
                "AllGather", ALU.bypass,
                replica_groups=[list(range(ncores))],
                ins=[loc_xc.opt()], outs=[tab.opt()])
            routing_layer(1, x3b, tab)

            # ---- Tj table ----
            if STAGE < 4:
                raise _StopTrace()
            nc.gpsimd.collective_compute(
                "AllGather", ALU.bypass,
                replica_groups=[list(range(ncores))],
                ins=[loc_tj.opt()], outs=[tab_tj.opt()])

            # ===================================================
            # Phase D: meta aggregation + mlp + log_softmax
            # ===================================================
            for t in range(NT):
                tg = zpool.tile([P, N4, CUT, D], BF, tag="z")
                nc.gpsimd.dma_gather(
                    out_ap=tg.rearrange("p n a d -> p (n a) d"),
                    in_ap=tab_tj[:],
                    idxs_ap=idxg[t][:],
                    num_idxs=NG, num_idxs_reg=NG, elem_size=D,
                    single_packet=False)
                prodm = zupool.tile([P, N4, CUT, C, K], F32, tag="zuf")
                nc.vector.tensor_tensor(
                    prodm[:],
                    tg[:, :, :, 0:C][:, :, :, :, None].broadcast_to([P, N4, CUT, C, K]),
                    ohs[t][:, :, :, None, :].broadcast_to([P, N4, CUT, C, K]),
                    op=ALU.mult)
                magg = wk.tile([P, N4, C, K], F32, tag="magg")
                nc.vector.tensor_reduce(
                    magg[:], prodm.rearrange("p n a c k -> p n c k a"),
                    axis=AX.X, op=ALU.add)
                meta = wk.tile([P, N4, C, K], F32, tag="meta")
                nc.vector.scalar_tensor_tensor(
                    meta[:], magg[:], 1.0 / (CUT * CUT), uo[t][:],
                    op0=ALU.mult, op1=ALU.add)
                nc.vector.tensor_scalar_max(meta[:], meta[:], 0.0)
                mo = wk.tile([P, N4, NCLASS], F32, tag="mo")
                for n4 in range(N4):
                    tp = ps.tile([P, P], F32, tag="tp")
                    nc.tensor.transpose(
                        tp[:], meta.rearrange("p n c k -> p n (c k)")[:, n4, :],
                        ident[:])
                    mT = wk.tile([P, P], F32, tag="mT")
                    nc.vector.tensor_copy(mT[:], tp[:])
                    mp = ps.tile([P, NCLASS], F32, tag="mp")
                    nc.tensor.matmul(mp[:], lhsT=mT[:], rhs=mlpw[:])
                    nc.vector.tensor_tensor(mo[:, n4, :], mp[:], mlpb[:], op=ALU.add)
                # log_softmax over classes
                mxc = sm.tile([P, N4], F32, tag="mxc")
                nc.vector.tensor_reduce(mxc[:], mo[:], axis=AX.X, op=ALU.max)
                sh = wk.tile([P, N4, NCLASS], F32, tag="sh")
                nc.vector.tensor_tensor(
                    sh[:], mo[:], mxc[:, :, None].broadcast_to([P, N4, NCLASS]),
                    op=ALU.subtract)
                ex = wk.tile([P, N4, NCLASS], F32, tag="ex")
                nc.scalar.activation(ex[:], sh[:], ACTF.Exp)
                se = sm.tile([P, N4], F32, tag="se")
                nc.vector.tensor_reduce(se[:], ex[:], axis=AX.X, op=ALU.add)
                lse = sm.tile([P, N4], F32, tag="lse")
                nc.scalar.activation(lse[:], se[:], ACTF.Ln)
                lsm = wk.tile([P, N4, NCLASS], F32, tag="lsm")
                nc.vector.tensor_tensor(
                    lsm[:], sh[:], lse[:, :, None].broadcast_to([P, N4, NCLASS]),
                    op=ALU.subtract)
                for n4 in range(N4):
                    cnt = _cnt(nloc, t, n4)
                    if cnt > 0:
                        r0 = t * TN + n4 * P
                        nc.sync.dma_start(out=out_meta[r0:r0 + cnt, :],
                                          in_=mo[0:cnt, n4, :])
                        nc.sync.dma_start(out=out_lsm[r0:r0 + cnt, :],
                                          in_=lsm[0:cnt, n4, :])

      except _StopTrace:
        pass
    nc.compile()
    return nc


# =======================================================================
# host side
# =======================================================================

def _prep_idx(arr_lin):
    """linear idx order -> dma_gather SBUF layout [128, N/16] int16"""
    n = arr_lin.shape[0]
    a = arr_lin.reshape(n // 16, 16).T.astype(np.int16)  # [16, n/16]
    return np.tile(a, (8, 1))


def _build_inputs(x, nbm, pca_w, pca_b, ln_g, ln_b, w_qs, w_ks, mlp_w, mlp_b,
                  ncores, nloc):
    NT = (nloc + TN - 1) // TN
    NZ = TN * M
    NG = TN * CUT
    bf = ml_dtypes.bfloat16

    mlp_w_perm = np.ascontiguousarray(
        mlp_w.reshape(K, C, NCLASS).transpose(1, 0, 2).reshape(D, NCLASS))
    common = {
        "pcaw": np.ascontiguousarray(pca_w, np.float32),
        "pcab": np.ascontiguousarray(pca_b, np.float32),
        "wqsT": np.ascontiguousarray(w_qs.T).astype(bf),
        "wksT": np.ascontiguousarray(w_ks.T).astype(bf),
        "lng": np.ascontiguousarray(ln_g, np.float32),
        "lnb": np.ascontiguousarray(ln_b, np.float32),
        "mlpw": mlp_w_perm.astype(np.float32),
        "mlpb": np.ascontiguousarray(mlp_b, np.float32),
        "ident": np.eye(P, dtype=np.float32),
        "tbv": np.arange(K, dtype=np.float32),
    }

    in_maps = []
    for cix in range(ncores):
        base = cix * nloc
        xl = np.ascontiguousarray(x[base:base + nloc], np.float32)
        idxz = np.zeros((NT, P, NZ // 16), np.int16)
        idxg = np.zeros((NT, P, NG // 16), np.int16)
        for t in range(NT):
            # z-gather: lin[(n4*M + m)*128 + p] = nbm[base + t*TN + n4*128 + p, m]
            lin = np.zeros((N4, M, P), np.int64)
            ling = np.zeros((N4, CUT, P), np.int64)
            for n4 in range(N4):
                cnt = _cnt(nloc, t, n4)
                if cnt == 0:
                    continue
                r0 = base + t * TN + n4 * P
                lin[n4, :, 0:cnt] = nbm[r0:r0 + cnt, :].T
                ling[n4, :, 0:cnt] = nbm[r0:r0 + cnt, 0:CUT].T
            idxz[t] = _prep_idx(lin.reshape(-1))
            idxg[t] = _prep_idx(ling.reshape(-1))
        m = dict(common)
        m.update({"x": xl, "idxz": idxz, "idxg": idxg})
        in_maps.append(m)
    return in_maps


def run(x, nb, pca_w, pca_b, ln_g, ln_b, w_qs, w_ks, mlp_w, mlp_b,
        ncores=NCORES, repeat_timing=0):
    n = x.shape[0]
    nloc = n // ncores
    assert nloc * ncores == n
    nbm = np.where(nb < 0, n, nb).astype(np.int64)

    key = (ncores, nloc)
    if key not in _CACHE:
        _CACHE[key] = build_program(ncores, nloc)
    nc = _CACHE[key]

    in_maps = _build_inputs(x, nbm, pca_w, pca_b, ln_g, ln_b, w_qs, w_ks,
                            mlp_w, mlp_b, ncores, nloc)
    res = run_bass_kernel_spmd(nc, in_maps, core_ids=list(range(ncores)))
    results = res.results

    lsm = np.concatenate([r["out_lsm"] for r in results], axis=0)
    meta = np.concatenate([r["out_meta"] for r in results], axis=0)
    attn_parts = np.concatenate([r["out_attn"][:, 0] for r in results], axis=0)
    off_sum = K * K - K
    attn_loss = np.float32(attn_parts.sum() / (off_sum * n))
    return lsm, attn_loss, meta


def kernel(x, nb, pca_w, pca_b, ln_g, ln_b, w_qs, w_ks, mlp_w, mlp_b):
    return run(np.asarray(x, np.float32), np.asarray(nb), np.asarray(pca_w),
               np.asarray(pca_b), np.asarray(ln_g), np.asarray(ln_b),
               np.asarray(w_qs), np.asarray(w_ks), np.asarray(mlp_w),
               np.asarray(mlp_b))


# revision 17
# speedup vs baseline: 15.1286x; 15.1286x over previous
"""Trainium2 Bass kernel for nn_CapsuleNet (gnn_message_passing).

Strategy (sharding_hint): shard nodes across the 8 NeuronCores (contiguous
blocks).  Each layer, every core normalizes its own nodes' features, the
per-core tables are AllGathered into a full bf16 node-feature table in DRAM,
and each core dma_gathers its local nodes' neighbor rows (z) once per layer.
The 6 dynamic-routing iterations, the argmax one-hot and the meta-path
aggregation are then fully node-local.

Layout: nodes on partitions, packed 4 per partition (tile = 512 nodes);
feature vectors stored (c,k)-permuted so the capsule axis k is innermost
(makes the p-broadcast multiply a unit-stride bf16 2x op).  The two big
routing einsums run as bf16 tensor_tensor multiplies + halving-tree adds.
rsqrt/normalize uses exp(-0.5*ln(x)) so the whole kernel stays on one ACT
table set (natural_log_exp_and_others).
"""
import os
import sys

sys.path.insert(0, '/opt/trn_rl_repo')

STAGE = int(os.environ.get("KSTAGE", "4"))
RSUB = int(os.environ.get("KRSUB", "9"))

import numpy as np
import ml_dtypes

import concourse.bass as bass
import concourse.bacc as bacc
import concourse.mybir as mybir
from concourse.tile import TileContext
from concourse.bass_utils import run_bass_kernel_spmd

dt = mybir.dt
AX = mybir.AxisListType
ALU = mybir.AluOpType
ACTF = mybir.ActivationFunctionType

# --- problem constants (hardcoded per spec) ---
K, C, D = 8, 16, 128          # capsules, hidden-per-capsule, d = K*C
M = 32                        # neighbors per node
F, FP = 500, 512              # input features (padded to 4x128)
NCLASS = 16
ROUTIT = 6
CUT = 5
NCORES = 8
N_TOTAL = 20000
NLOC = N_TOTAL // NCORES      # 2500 nodes per core
P = 128                       # partitions
N4 = 2                        # nodes per partition per tile
TN = P * N4                   # 512 nodes per tile
BF = dt.bfloat16
F32 = dt.float32

_CACHE = {}


class _StopTrace(Exception):
    pass


def _cnt(nloc, t, n4):
    """valid node count for (tile, n4) block"""
    return max(0, min(128, nloc - (t * TN + n4 * P)))


def build_program(ncores, nloc):
    NT = (nloc + TN - 1) // TN
    n_total = ncores * nloc
    NZ = TN * M               # z-gather indices per tile
    NG = TN * CUT             # meta-gather indices per tile

    nc = bacc.Bacc("TRN2", target_bir_lowering=False, debug=False,
                   num_devices=ncores)

    # ---- I/O ----
    x_in = nc.dram_tensor("x", [nloc, F], F32, kind="ExternalInput")
    idxz_in = nc.dram_tensor("idxz", [NT, P, NZ // 16], dt.int16, kind="ExternalInput")
    idxg_in = nc.dram_tensor("idxg", [NT, P, NG // 16], dt.int16, kind="ExternalInput")
    pcaw_in = nc.dram_tensor("pcaw", [F, D], F32, kind="ExternalInput")
    pcab_in = nc.dram_tensor("pcab", [D], F32, kind="ExternalInput")
    wqsT_in = nc.dram_tensor("wqsT", [C, C], BF, kind="ExternalInput")
    wksT_in = nc.dram_tensor("wksT", [C, C], BF, kind="ExternalInput")
    lng_in = nc.dram_tensor("lng", [C], F32, kind="ExternalInput")
    lnb_in = nc.dram_tensor("lnb", [C], F32, kind="ExternalInput")
    mlpw_in = nc.dram_tensor("mlpw", [D, NCLASS], F32, kind="ExternalInput")  # (c,k)-permuted rows
    mlpb_in = nc.dram_tensor("mlpb", [NCLASS], F32, kind="ExternalInput")
    ident_in = nc.dram_tensor("ident", [P, P], F32, kind="ExternalInput")
    tbv_in = nc.dram_tensor("tbv", [K], F32, kind="ExternalInput")

    out_lsm = nc.dram_tensor("out_lsm", [nloc, NCLASS], F32, kind="ExternalOutput")
    out_meta = nc.dram_tensor("out_meta", [nloc, NCLASS], F32, kind="ExternalOutput")
    out_attn = nc.dram_tensor("out_attn", [nloc, 1], F32, kind="ExternalOutput")

    with TileContext(nc) as tc:
      try:
        with (
            tc.tile_pool(name="consts", bufs=1) as cp,
            tc.tile_pool(name="res", bufs=1) as rp,
            tc.tile_pool(name="zp", bufs=2) as zpool,
            tc.tile_pool(name="zu", bufs=1) as zupool,
            tc.tile_pool(name="wk2", bufs=2) as wk2,
            tc.tile_pool(name="wk", bufs=1) as wk,
            tc.tile_pool(name="sm", bufs=1) as sm,
            tc.tile_pool(name="psum", bufs=2, space="PSUM") as ps,
            tc.tile_pool(name="dram", bufs=1, space="DRAM") as dram,
        ):
            # ---------------- constants ----------------
            ident = cp.tile([P, P], F32)
            nc.sync.dma_start(out=ident[:], in_=ident_in[:, :])
            wt = cp.tile([P, 4, D], F32)          # pca_w f-chunks
            nc.gpsimd.memset(wt[:], 0.0)
            nc.sync.dma_start(out=wt[:, 0:3, :],
                              in_=pcaw_in[0:384, :].rearrange("(c p) d -> p c d", p=P))
            nc.sync.dma_start(out=wt[0:F - 384, 3, :], in_=pcaw_in[384:F, :])
            bt = cp.tile([P, D], F32)
            nc.sync.dma_start(out=bt[:], in_=pcab_in[None, :].to_broadcast([P, D]))
            wqsT = cp.tile([P, C, C], BF)
            nc.sync.dma_start(out=wqsT[:], in_=wqsT_in[None, :, :].to_broadcast([P, C, C]))
            wksT = cp.tile([P, C, C], BF)
            nc.sync.dma_start(out=wksT[:], in_=wksT_in[None, :, :].to_broadcast([P, C, C]))
            lng = cp.tile([P, C], F32)
            nc.sync.dma_start(out=lng[:], in_=lng_in[None, :].to_broadcast([P, C]))
            lnb = cp.tile([P, C], F32)
            nc.sync.dma_start(out=lnb[:], in_=lnb_in[None, :].to_broadcast([P, C]))
            mlpw = cp.tile([P, NCLASS], F32)
            nc.sync.dma_start(out=mlpw[:], in_=mlpw_in[:, :])
            eps6 = cp.tile([P, 1], F32)
            nc.gpsimd.memset(eps6[:], 1e-6)
            kidx = cp.tile([P, K], F32)
            nc.sync.dma_start(out=kidx[:], in_=tbv_in[None, :].to_broadcast([P, K]))
            kplus = cp.tile([P, K], F32)
            nc.vector.tensor_scalar_add(kplus[:], kidx[:], 999.0)
            mlpb = cp.tile([P, NCLASS], F32)
            nc.sync.dma_start(out=mlpb[:], in_=mlpb_in[None, :].to_broadcast([P, NCLASS]))
            idxz = []
            idxg = []
            for t in range(NT):
                iz = cp.tile([P, NZ // 16], dt.int16, tag=f"idxz{t}", name=f"idxz{t}")
                nc.sync.dma_start(out=iz[:], in_=idxz_in[t, :, :])
                idxz.append(iz)
                ig = cp.tile([P, NG // 16], dt.int16, tag=f"idxg{t}")
                nc.sync.dma_start(out=ig[:], in_=idxg_in[t, :, :])
                idxg.append(ig)

            # ---------------- DRAM tables ----------------
            loc_xc = dram.tile([nloc, D], F32)    # this core's x_c rows (per layer)
            tab = dram.tile([n_total, D], F32)    # gathered full table (per layer)
            loc_tj = dram.tile([nloc, D], BF)     # padded Tj~ rows
            tab_tj = dram.tile([n_total, D], BF)

            # ---------------- residents ----------------
            x3a = [rp.tile([P, N4, C, K], F32, tag=f"x3a{t}", name=f"x3a{t}") for t in range(NT)]
            x3b = [rp.tile([P, N4, C, K], F32, tag=f"x3b{t}", name=f"x3b{t}") for t in range(NT)]
            uo = [rp.tile([P, N4, C, K], F32, tag=f"uo{t}", name=f"uo{t}") for t in range(NT)]
            ohs = [rp.tile([P, N4, CUT, K], BF, tag=f"oh{t}", name=f"oh{t}") for t in range(NT)]

            # ===================================================
            # helpers
            # ===================================================
            def normalize(u, rn_out_tag):
                """rn = 1/max(||u||_c, 1e-12) per (n4,k); returns rn [P,N4,K] f32"""
                usq = sm.tile([P, N4, C, K], F32, tag="usq")
                nc.scalar.activation(usq[:], u[:], ACTF.Square)
                ss = sm.tile([P, N4, K], F32, tag=rn_out_tag + "ss")
                nc.vector.tensor_reduce(
                    ss[:], usq.rearrange("p n c k -> p n k c"),
                    axis=AX.X, op=ALU.add)
                nc.vector.tensor_scalar_max(ss[:], ss[:], 1e-24)
                lg = sm.tile([P, N4, K], F32, tag=rn_out_tag + "lg")
                nc.scalar.activation(lg[:], ss[:], ACTF.Ln)
                rn = sm.tile([P, N4, K], F32, tag=rn_out_tag)
                nc.scalar.activation(rn[:], lg[:], ACTF.Exp, scale=-0.5)
                # one Newton step: rn *= (1.5 - 0.5*ss*rn^2)
                t1 = sm.tile([P, N4, K], F32, tag=rn_out_tag + "t1")
                nc.vector.tensor_tensor(t1[:], rn[:], rn[:], op=ALU.mult)
                nc.vector.tensor_tensor(t1[:], t1[:], ss[:], op=ALU.mult)
                nc.vector.tensor_scalar(t1[:], t1[:], -0.5, 1.5,
                                        op0=ALU.mult, op1=ALU.add)
                nc.vector.tensor_tensor(rn[:], rn[:], t1[:], op=ALU.mult)
                return rn

            # ===================================================
            # Phase A: pca + attention + x3a + x_c table rows
            # ===================================================
            for t in range(NT):
                xt = wk2.tile([P, N4, FP], F32, tag="xt")
                if _cnt(nloc, t, N4 - 1) < P:
                    nc.gpsimd.memset(xt[:], 0.0)
                else:
                    nc.gpsimd.memset(xt[:, :, F:FP], 0.0)
                for n4 in range(N4):
                    cnt = _cnt(nloc, t, n4)
                    if cnt > 0:
                        r0 = t * TN + n4 * P
                        nc.sync.dma_start(out=xt[0:cnt, n4, 0:F],
                                          in_=x_in[r0:r0 + cnt, :])
                # pca matmul -> h [p, n4, (k,c)]
                h = wk.tile([P, N4, K, C], F32, tag="h")
                for n4 in range(N4):
                    xTs = wk.tile([P, 4, P], F32, tag="xTs")
                    for ci in range(4):
                        tp = ps.tile([P, P], F32, tag="tp")
                        nc.tensor.transpose(
                            tp[:], xt[:, n4, ci * P:(ci + 1) * P], ident[:])
                        nc.vector.tensor_copy(xTs[:, ci, :], tp[:])
                    hp = ps.tile([P, D], F32, tag="hp")
                    for ci in range(4):
                        nc.tensor.matmul(hp[:], lhsT=xTs[:, ci, :], rhs=wt[:, ci, :],
                                         start=(ci == 0), stop=(ci == 3))
                    nc.vector.tensor_tensor(
                        h.rearrange("p n k c -> p n (k c)")[:, n4, :],
                        hp[:], bt[:], op=ALU.add)

                # ---- independence attention (pre-relu h3) ----
                mu = sm.tile([P, N4, K], F32, tag="mu")
                nc.vector.tensor_reduce(mu[:], h[:], axis=AX.X, op=ALU.add)
                nc.vector.tensor_scalar_mul(mu[:], mu[:], 1.0 / C)
                hm = wk.tile([P, N4, K, C], F32, tag="hm")
                nc.vector.tensor_tensor(
                    hm[:], h[:], mu[:, :, :, None].broadcast_to([P, N4, K, C]),
                    op=ALU.subtract)
                hsq = wk.tile([P, N4, K, C], F32, tag="usq2")
                nc.scalar.activation(hsq[:], hm[:], ACTF.Square)
                ssq = sm.tile([P, N4, K], F32, tag="ssq")
                nc.vector.tensor_reduce(ssq[:], hsq[:], axis=AX.X, op=ALU.add)
                # rstd = exp(-0.5*ln(ssq/16 + 1e-6))
                lgv = sm.tile([P, N4, K], F32, tag="lgv")
                nc.scalar.activation(lgv[:], ssq[:], ACTF.Ln, scale=1.0 / C, bias=eps6[:])
                rstd = sm.tile([P, N4, K], F32, tag="rstd")
                nc.scalar.activation(rstd[:], lgv[:], ACTF.Exp, scale=-0.5)
                ln = wk.tile([P, N4, K, C], F32, tag="ln")
                nc.vector.tensor_tensor(
                    ln[:], hm[:], rstd[:, :, :, None].broadcast_to([P, N4, K, C]),
                    op=ALU.mult)
                nc.vector.tensor_tensor(
                    ln[:], ln[:],
                    lng[:, None, None, :].broadcast_to([P, N4, K, C]), op=ALU.mult)
                nc.vector.tensor_tensor(
                    ln[:], ln[:],
                    lnb[:, None, None, :].broadcast_to([P, N4, K, C]), op=ALU.add)
                lnb16 = wk.tile([P, N4, K, C], BF, tag="lnb16")
                nc.vector.tensor_copy(lnb16[:], ln[:])
                hb16 = wk.tile([P, N4, K, C], BF, tag="hb16")
                nc.vector.tensor_copy(hb16[:], h[:])

                # q = LN @ wqs, kk = h3 @ wks  (per-capsule shared 16x16)
                prod = zupool.tile([P, N4, K, C, C], BF, tag="zu")
                q = wk.tile([P, N4, K, C], BF, tag="q")
                kk = wk.tile([P, N4, K, C], BF, tag="kk")
                for (src, wT, dst) in ((lnb16, wqsT, q), (hb16, wksT, kk)):
                    nc.vector.tensor_tensor(
                        prod[:],
                        src[:, :, :, None, :].broadcast_to([P, N4, K, C, C]),
                        wT[:, None, None, :, :].broadcast_to([P, N4, K, C, C]),
                        op=ALU.mult)
                    cs = C // 2
                    while cs >= 2:
                        nc.vector.tensor_tensor(
                            prod[:, :, :, :, 0:cs], prod[:, :, :, :, 0:cs],
                            prod[:, :, :, :, cs:2 * cs], op=ALU.add)
                        cs //= 2
                    nc.vector.tensor_tensor(
                        dst[:], prod[:, :, :, :, 0], prod[:, :, :, :, 1], op=ALU.add)
                # logits l[kq, kk'] = sum_c q[kq,c]*kk[kk',c]
                prodl = zupool.tile([P, N4, K, K, C], BF, tag="zu")
                nc.vector.tensor_tensor(
                    prodl[:],
                    q[:, :, :, None, :].broadcast_to([P, N4, K, K, C]),
                    kk[:, :, None, :, :].broadcast_to([P, N4, K, K, C]),
                    op=ALU.mult)
                cs = C // 2
                while cs >= 2:
                    nc.vector.tensor_tensor(
                        prodl[:, :, :, :, 0:cs], prodl[:, :, :, :, 0:cs],
                        prodl[:, :, :, :, cs:2 * cs], op=ALU.add)
                    cs //= 2
                lgt = wk.tile([P, N4, K, K], F32, tag="lgt")
                nc.vector.tensor_tensor(
                    lgt[:], prodl[:, :, :, :, 0], prodl[:, :, :, :, 1], op=ALU.add)
                pel = wk.tile([P, N4, K, K], F32, tag="pel")
                nc.scalar.activation(pel[:], lgt[:], ACTF.Exp, scale=0.25)  # /temp=4
                den = sm.tile([P, N4, K], F32, tag="den")
                nc.vector.tensor_reduce(den[:], pel[:], axis=AX.X, op=ALU.add)
                rden = sm.tile([P, N4, K], F32, tag="rden")
                nc.vector.reciprocal(rden[:], den[:])
                dg = sm.tile([P, N4, K], F32, tag="dg")
                nc.vector.tensor_tensor(
                    dg[:],
                    pel.rearrange("p n a b -> p n (a b)")[:, :, 0:K * K:K + 1],
                    rden[:], op=ALU.mult)
                sdg = sm.tile([P, N4], F32, tag="sdg")
                nc.vector.tensor_reduce(sdg[:], dg[:], axis=AX.X, op=ALU.add)
                att = sm.tile([P, N4], F32, tag="att")
                nc.vector.tensor_scalar(att[:], sdg[:], -1.0, float(K),
                                        op0=ALU.mult, op1=ALU.add)
                for n4 in range(N4):
                    cnt = _cnt(nloc, t, n4)
                    if cnt > 0:
                        r0 = t * TN + n4 * P
                        nc.sync.dma_start(out=out_attn[r0:r0 + cnt, :],
                                          in_=att[0:cnt, n4, None])

                # ---- x0 = relu(h); x3a = normalize(x0) stored (c,k) ----
                x0 = wk.tile([P, N4, K, C], F32, tag="x0")
                nc.vector.tensor_scalar_max(x0[:], h[:], 0.0)
                x0sq = wk.tile([P, N4, K, C], F32, tag="usq2")
                nc.scalar.activation(x0sq[:], x0[:], ACTF.Square)
                ss0 = sm.tile([P, N4, K], F32, tag="ss0")
                nc.vector.tensor_reduce(ss0[:], x0sq[:], axis=AX.X, op=ALU.add)
                nc.vector.tensor_scalar_max(ss0[:], ss0[:], 1e-24)
                lg0 = sm.tile([P, N4, K], F32, tag="lg0")
                nc.scalar.activation(lg0[:], ss0[:], ACTF.Ln)
                rn0 = sm.tile([P, N4, K], F32, tag="rn0")
                nc.scalar.activation(rn0[:], lg0[:], ACTF.Exp, scale=-0.5)
                t10 = sm.tile([P, N4, K], F32, tag="t10")
                nc.vector.tensor_tensor(t10[:], rn0[:], rn0[:], op=ALU.mult)
                nc.vector.tensor_tensor(t10[:], t10[:], ss0[:], op=ALU.mult)
                nc.vector.tensor_scalar(t10[:], t10[:], -0.5, 1.5,
                                        op0=ALU.mult, op1=ALU.add)
                nc.vector.tensor_tensor(rn0[:], rn0[:], t10[:], op=ALU.mult)
                # permuted write: x3a[(c,k)] = x0[(k,c)] * rn0[k]
                nc.vector.tensor_tensor(
                    x3a[t].rearrange("p n c k -> p n k c"),
                    x0[:], rn0[:, :, :, None].broadcast_to([P, N4, K, C]),
                    op=ALU.mult)
                for n4 in range(N4):
                    cnt = _cnt(nloc, t, n4)
                    if cnt > 0:
                        r0 = t * TN + n4 * P
                        nc.sync.dma_start(
                            out=loc_xc[r0:r0 + cnt, :],
                            in_=x3a[t].rearrange("p n c k -> p n (c k)")[0:cnt, n4, :])

            # ===================================================
            # routing layer runner
            # ===================================================
            def routing_layer(layer, x3, tab_ap):
                """returns final u per tile (f32 (c,k)); last layer also p_raw5+z kept"""
                for t in range(NT):
                    if RSUB == 0:
                        continue
                    z = zpool.tile([P, N4, M, D], F32, tag="z")
                    nc.gpsimd.dma_gather(
                        out_ap=z.rearrange("p n m d -> p (n m) d"),
                        in_ap=tab_ap[:],
                        idxs_ap=idxz[t][:],
                        num_idxs=NZ, num_idxs_reg=NZ, elem_size=D,
                        single_packet=False)
                    z5 = z.rearrange("p n m (c k) -> p n m c k", c=C)
                    if RSUB == 1:
                        continue

                    u = sm.tile([P, N4, C, K], F32, tag="u")
                    zu = zupool.tile([P, N4, M, D], F32, tag="zu")
                    zu5 = zu.rearrange("p n m (c k) -> p n m c k", c=C)

                    # it0: u = (1/8) sum_m z + x3
                    usum = sm.tile([P, N4, C, K], F32, tag="usum")
                    nc.vector.tensor_reduce(
                        usum.rearrange("p n c k -> p n (c k)"),
                        z.rearrange("p n m d -> p n d m"),
                        axis=AX.X, op=ALU.add)
                    nc.vector.scalar_tensor_tensor(
                        u[:], usum[:], 1.0 / K, x3[t][:],
                        op0=ALU.mult, op1=ALU.add)
                    rn = normalize(u, "rnit")
                    un = sm.tile([P, N4, C, K], F32, tag="un")
                    nc.vector.tensor_tensor(
                        un[:], u[:], rn[:, :, None, :].broadcast_to([P, N4, C, K]),
                        op=ALU.mult)

                    praw = sm.tile([P, N4, M, K], F32, tag="praw")
                    if RSUB == 2:
                        continue
                    for it in range(1, min(ROUTIT, RSUB)):
                        last_it = it == ROUTIT - 1
                        # p = softmax_k(z . u)
                        nc.vector.tensor_tensor(
                            zu[:], z[:],
                            un.rearrange("p n c k -> p n (c k)")[:, :, None, :]
                              .broadcast_to([P, N4, M, D]),
                            op=ALU.mult)
                        nc.vector.tensor_reduce(
                            praw[:], zu5.rearrange("p n m c k -> p n m k c"),
                            axis=AX.X, op=ALU.add)
                        pe = sm.tile([P, N4, M, K], F32, tag="pe")
                        nc.scalar.activation(pe[:], praw[:], ACTF.Exp)
                        den = sm.tile([P, N4, M], F32, tag="denr")
                        nc.vector.tensor_reduce(den[:], pe[:], axis=AX.X, op=ALU.add)
                        rec = sm.tile([P, N4, M], F32, tag="rec")
                        nc.vector.reciprocal(rec[:], den[:])
                        pn = sm.tile([P, N4, M, K], F32, tag="pn")
                        nc.vector.tensor_tensor(
                            pn[:], pe[:],
                            rec[:, :, :, None].broadcast_to([P, N4, M, K]),
                            op=ALU.mult)
                        # u = sum_m pn * z + x3
                        nc.vector.tensor_tensor(
                            zu5[:], z5[:],
                            pn[:, :, :, None, :].broadcast_to([P, N4, M, C, K]),
                            op=ALU.mult)
                        nc.vector.tensor_reduce(
                            u[:], zu5.rearrange("p n m c k -> p n c k m"),
                            axis=AX.X, op=ALU.add)
                        nc.vector.tensor_tensor(u[:], u[:], x3[t][:], op=ALU.add)
                        if not last_it:
                            rn = normalize(u, "rnit")
                            nc.vector.tensor_tensor(
                                un[:], u[:],
                                rn[:, :, None, :].broadcast_to([P, N4, C, K]),
                                op=ALU.mult)

                    if layer == 0:
                        # h1 = relu(u); x3b = normalize(h1); write x_c rows
                        x1 = sm.tile([P, N4, C, K], F32, tag="x1")
                        nc.vector.tensor_scalar_max(x1[:], u[:], 0.0)
                        x1sq = sm.tile([P, N4, C, K], F32, tag="usq")
                        nc.scalar.activation(x1sq[:], x1[:], ACTF.Square)
                        ss1 = sm.tile([P, N4, K], F32, tag="ss1")
                        nc.vector.tensor_reduce(
                            ss1[:], x1sq.rearrange("p n c k -> p n k c"),
                            axis=AX.X, op=ALU.add)
                        nc.vector.tensor_scalar_max(ss1[:], ss1[:], 1e-24)
                        lg1 = sm.tile([P, N4, K], F32, tag="lg1")
                        nc.scalar.activation(lg1[:], ss1[:], ACTF.Ln)
                        rn1 = sm.tile([P, N4, K], F32, tag="rn1")
                        nc.scalar.activation(rn1[:], lg1[:], ACTF.Exp, scale=-0.5)
                        t11 = sm.tile([P, N4, K], F32, tag="t11")
                        nc.vector.tensor_tensor(t11[:], rn1[:], rn1[:], op=ALU.mult)
                        nc.vector.tensor_tensor(t11[:], t11[:], ss1[:], op=ALU.mult)
                        nc.vector.tensor_scalar(t11[:], t11[:], -0.5, 1.5,
                                                op0=ALU.mult, op1=ALU.add)
                        nc.vector.tensor_tensor(rn1[:], rn1[:], t11[:], op=ALU.mult)
                        nc.vector.tensor_tensor(
                            x3b[t][:], x1[:],
                            rn1[:, :, None, :].broadcast_to([P, N4, C, K]),
                            op=ALU.mult)
                        for n4 in range(N4):
                            cnt = _cnt(nloc, t, n4)
                            if cnt > 0:
                                r0 = t * TN + n4 * P
                                nc.sync.dma_start(
                                    out=loc_xc[r0:r0 + cnt, :],
                                    in_=x3b[t].rearrange("p n c k -> p n (c k)")[0:cnt, n4, :])
                    else:
                        # keep u; one-hot argmax of first CUT slots; Tj~ rows
                        nc.vector.tensor_copy(uo[t][:], u[:])
                        p5 = praw[:, :, 0:CUT, :]
                        mx = sm.tile([P, N4, CUT], F32, tag="mx5")
                        nc.vector.tensor_reduce(mx[:], p5, axis=AX.X, op=ALU.max)
                        ohm = sm.tile([P, N4, CUT, K], F32, tag="ohm")
                        nc.vector.tensor_tensor(
                            ohm[:], p5,
                            mx[:, :, :, None].broadcast_to([P, N4, CUT, K]),
                            op=ALU.is_ge)
                        # exact first-max: min k among maxima
                        cand = sm.tile([P, N4, CUT, K], F32, tag="cand")
                        nc.vector.scalar_tensor_tensor(
                            cand.rearrange("p n a k -> p (n a) k"),
                            ohm.rearrange("p n a k -> p (n a) k"), -999.0,
                            kplus[:, None, :].broadcast_to([P, N4 * CUT, K]),
                            op0=ALU.mult, op1=ALU.add)
                        mnk = sm.tile([P, N4, CUT], F32, tag="mnk")
                        nc.vector.tensor_reduce(mnk[:], cand[:], axis=AX.X, op=ALU.min)
                        nc.vector.tensor_tensor(
                            ohs[t][:],
                            kidx[:, None, None, :].broadcast_to([P, N4, CUT, K]),
                            mnk[:, :, :, None].broadcast_to([P, N4, CUT, K]),
                            op=ALU.is_equal)
                        # Tj~[c] = sum_{a<CUT} sum_k oh[a,k] * z[a,c,k]
                        zz = zupool.tile([P, N4, CUT, C, K], BF, tag="zu")
                        nc.vector.tensor_tensor(
                            zz[:], z5[:, :, 0:CUT, :, :],
                            ohs[t][:, :, :, None, :].broadcast_to([P, N4, CUT, C, K]),
                            op=ALU.mult)
                        tj = sm.tile([P, N4, C], F32, tag="tj")
                        nc.vector.tensor_reduce(
                            tj[:], zz.rearrange("p n a c k -> p n c a k"),
                            axis=AX.XY, op=ALU.add)
                        tjb = sm.tile([P, N4, C], BF, tag="tjb")
                        nc.vector.tensor_copy(tjb[:], tj[:])
                        for n4 in range(N4):
                            cnt = _cnt(nloc, t, n4)
                            if cnt > 0:
                                r0 = t * TN + n4 * P
                                nc.sync.dma_start(out=loc_tj[r0:r0 + cnt, 0:C],
                                                  in_=tjb[0:cnt, n4, :])

            # ---- layer 0 ----
            if STAGE < 2:
                raise _StopTrace()
            nc.gpsimd.collective_compute(
                "AllGather", ALU.bypass,
                replica_groups=[list(range(ncores))],
                ins=[loc_xc.opt()], outs=[tab.opt()])
            routing_layer(0, x3a, tab)

            # ---- layer 1 ----
            if STAGE < 3:
                raise _StopTrace()
            nc.gpsimd.collective_compute(
                "AllGather", ALU.bypass,
                replica_groups=[list(range(ncores))],
                ins=[loc_xc.opt()], outs=[tab.opt()])
            routing_layer(1, x3b, tab)

            # ---- Tj table ----
            if STAGE < 4:
                raise _StopTrace()
            nc.gpsimd.collective_compute(
                "AllGather", ALU.bypass,
                replica_groups=[list(range(ncores))],
                ins=[loc_tj.opt()], outs=[tab_tj.opt()])

            # ===================================================
            # Phase D: meta aggregation + mlp + log_softmax
            # ===================================================
            for t in range(NT):
                tg = zpool.tile([P, N4, CUT, D], BF, tag="z")
                nc.gpsimd.dma_gather(
                    out_ap=tg.rearrange("p n a d -> p (n a) d"),
                    in_ap=tab_tj[:],
                    idxs_ap=idxg[t][:],
                    num_idxs=NG, num_idxs_reg=NG, elem_size=D,
                    single_packet=False)
                prodm = zupool.tile([P, N4, CUT, C, K], F32, tag="zuf")
                nc.vector.tensor_tensor(
                    prodm[:],
                    tg[:, :, :, 0:C][:, :, :, :, None].broadcast_to([P, N4, CUT, C, K]),
                    ohs[t][:, :, :, None, :].broadcast_to([P, N4, CUT, C, K]),
                    op=ALU.mult)
                magg = wk.tile([P, N4, C, K], F32, tag="magg")
                nc.vector.tensor_reduce(
                    magg[:], prodm.rearrange("p n a c k -> p n c k a"),
                    axis=AX.X, op=ALU.add)
                meta = wk.tile([P, N4, C, K], F32, tag="meta")
                nc.vector.scalar_tensor_tensor(
                    meta[:], magg[:], 1.0 / (CUT * CUT), uo[t][:],
                    op0=ALU.mult, op1=ALU.add)
                nc.vector.tensor_scalar_max(meta[:], meta[:], 0.0)
                mo = wk.tile([P, N4, NCLASS], F32, tag="mo")
                for n4 in range(N4):
                    tp = ps.tile([P, P], F32, tag="tp")
                    nc.tensor.transpose(
                        tp[:], meta.rearrange("p n c k -> p n (c k)")[:, n4, :],
                        ident[:])
                    mT = wk.tile([P, P], F32, tag="mT")
                    nc.vector.tensor_copy(mT[:], tp[:])
                    mp = ps.tile([P, NCLASS], F32, tag="mp")
                    nc.tensor.matmul(mp[:], lhsT=mT[:], rhs=mlpw[:])
                    nc.vector.tensor_tensor(mo[:, n4, :], mp[:], mlpb[:], op=ALU.add)
                # log_softmax over classes
                mxc = sm.tile([P, N4], F32, tag="mxc")
                nc.vector.tensor_reduce(mxc[:], mo[:], axis=AX.X, op=ALU.max)
                sh = wk.tile([P, N4, NCLASS], F32, tag="sh")
                nc.vector.tensor_tensor(
                    sh[:], mo[:], mxc[:, :, None].broadcast_to([P, N4, NCLASS]),
                    op=ALU.subtract)
                ex = wk.tile([P, N4, NCLASS], F32, tag="ex")
                nc.scalar.activation(ex[:], sh[:], ACTF.Exp)
                se = sm.tile([P, N4], F32, tag="se")
                nc.vector.tensor_reduce(se[:], ex[:], axis=AX.X, op=ALU.add)
                lse = sm.tile([P, N4], F32, tag="lse")
                nc.scalar.activation(lse[:], se[:], ACTF.Ln)
                lsm = wk.tile([P, N4, NCLASS], F32, tag="lsm")
                nc.vector.tensor_tensor(
                    lsm[:], sh[:], lse[:, :, None].broadcast_to([P, N4, NCLASS]),
                    op=ALU.subtract)
                for n4 in range(N4):
                    cnt = _cnt(nloc, t, n4)
                    if cnt > 0:
                        r0 = t * TN + n4 * P
                        nc.sync.dma_start(out=out_meta[r0:r0 + cnt, :],
                                          in_=mo[0:cnt, n4, :])
                        nc.sync.dma_start(out=out_lsm[r0:r0 + cnt, :],
                                          in_=lsm[0:cnt, n4, :])

      except _StopTrace:
        pass
    nc.compile()
    return nc


# =======================================================================
# host side
# =======================================================================

def _prep_idx(arr_lin):
    """linear idx order -> dma_gather SBUF layout [128, N/16] int16"""
    n = arr_lin.shape[0]
    a = arr_lin.reshape(n // 16, 16).T.astype(np.int16)  # [16, n/16]
    return np.tile(a, (8, 1))


def _build_inputs(x, nbm, pca_w, pca_b, ln_g, ln_b, w_qs, w_ks, mlp_w, mlp_b,
                  ncores, nloc):
    NT = (nloc + TN - 1) // TN
    NZ = TN * M
    NG = TN * CUT
    bf = ml_dtypes.bfloat16

    mlp_w_perm = np.ascontiguousarray(
        mlp_w.reshape(K, C, NCLASS).transpose(1, 0, 2).reshape(D, NCLASS))
    common = {
        "pcaw": np.ascontiguousarray(pca_w, np.float32),
        "pcab": np.ascontiguousarray(pca_b, np.float32),
        "wqsT": np.ascontiguousarray(w_qs.T).astype(bf),
        "wksT": np.ascontiguousarray(w_ks.T).astype(bf),
        "lng": np.ascontiguousarray(ln_g, np.float32),
        "lnb": np.ascontiguousarray(ln_b, np.float32),
        "mlpw": mlp_w_perm.astype(np.float32),
        "mlpb": np.ascontiguousarray(mlp_b, np.float32),
        "ident": np.eye(P, dtype=np.float32),
        "tbv": np.arange(K, dtype=np.float32),
    }

    in_maps = []
    for cix in range(ncores):
        base = cix * nloc
        xl = np.ascontiguousarray(x[base:base + nloc], np.float32)
        idxz = np.zeros((NT, P, NZ // 16), np.int16)
        idxg = np.zeros((NT, P, NG // 16), np.int16)
        for t in range(NT):
            # z-gather: lin[(n4*M + m)*128 + p] = nbm[base + t*TN + n4*128 + p, m]
            lin = np.zeros((N4, M, P), np.int64)
            ling = np.zeros((N4, CUT, P), np.int64)
            for n4 in range(N4):
                cnt = _cnt(nloc, t, n4)
                if cnt == 0:
                    continue
                r0 = base + t * TN + n4 * P
                lin[n4, :, 0:cnt] = nbm[r0:r0 + cnt, :].T
                ling[n4, :, 0:cnt] = nbm[r0:r0 + cnt, 0:CUT].T
            idxz[t] = _prep_idx(lin.reshape(-1))
            idxg[t] = _prep_idx(ling.reshape(-1))
        m = dict(common)
        m.update({"x": xl, "idxz": idxz, "idxg": idxg})
        in_maps.append(m)
    return in_maps


class PJRTRunner:
    """Persistent multi-core PJRT executable for a built Bass program.
    Mirrors bass2jax.run_bass_via_pjrt's multi-core branch, but keeps the
    jitted shard_map callable and device-resident inputs so repeated calls
    measure device execution without retracing/recompiling."""

    def __init__(self, nc, ncores):
        import jax
        from jax.sharding import Mesh, PartitionSpec
        from jax.experimental.shard_map import shard_map
        from concourse import bass2jax as b2j

        b2j.install_neuronx_cc_hook()
        self.ncores = ncores
        self.nc = nc
        in_names, out_names, out_avals, zero_shapes = [], [], [], []
        partition_name = nc.partition_id_tensor.name if nc.partition_id_tensor else None
        for alloc in nc.m.functions[0].allocations:
            if not isinstance(alloc, mybir.MemoryLocationSet):
                continue
            name = alloc.memorylocations[0].name
            if alloc.kind == "ExternalInput":
                if name != partition_name:
                    in_names.append(name)
            elif alloc.kind == "ExternalOutput":
                out_avals.append(jax.core.ShapedArray(
                    tuple(alloc.tensor_shape), mybir.dt.np(alloc.dtype)))
                out_names.append(name)
        self.in_names = list(in_names)
        self.out_names = out_names
        self.out_avals = out_avals
        n_params = len(in_names)
        n_outs = len(out_names)
        all_in_names = list(in_names) + list(out_names)
        if partition_name is not None:
            all_in_names.append(partition_name)
        donate = tuple(range(n_params, n_params + n_outs))

        def _body(*args):
            operands = list(args)
            if partition_name is not None:
                operands.append(b2j.partition_id_tensor())
            outs = b2j._bass_exec_p.bind(
                *operands,
                out_avals=tuple(out_avals),
                in_names=tuple(all_in_names),
                out_names=tuple(out_names),
                lowering_input_output_aliases=(),
                sim_require_finite=True,
                sim_require_nnan=True,
                nc=nc,
            )
            return tuple(outs)

        devices = jax.devices()[:ncores]
        self.mesh = Mesh(np.asarray(devices), ("core",))
        in_specs = (PartitionSpec("core"),) * (n_params + n_outs)
        out_specs = (PartitionSpec("core"),) * n_outs
        self.fn = jax.jit(
            shard_map(_body, mesh=self.mesh, in_specs=in_specs,
                      out_specs=out_specs, check_rep=False),
            donate_argnums=donate, keep_unused=True)
        self.n_params = n_params
        self.n_outs = n_outs

    def put_inputs(self, in_maps):
        import jax
        from jax.sharding import NamedSharding, PartitionSpec
        sh = NamedSharding(self.mesh, PartitionSpec("core"))
        concat = [np.concatenate([np.asarray(m[name]) for m in in_maps], axis=0)
                  for name in self.in_names]
        return [jax.device_put(a, sh) for a in concat]

    def zeros(self):
        import jax
        from jax.sharding import NamedSharding, PartitionSpec
        sh = NamedSharding(self.mesh, PartitionSpec("core"))
        return [jax.device_put(
                    np.zeros((self.ncores * av.shape[0], *av.shape[1:]), av.dtype), sh)
                for av in self.out_avals]

    def __call__(self, dev_inputs):
        outs = self.fn(*dev_inputs, *self.zeros())
        import jax
        jax.block_until_ready(outs)
        return outs

    def unpack(self, outs):
        res = []
        for c in range(self.ncores):
            res.append({name: np.asarray(outs[i]).reshape(
                            self.ncores, *self.out_avals[i].shape)[c]
                        for i, name in enumerate(self.out_names)})
        return res


_RUNNER = {}
LAST_EXEC_NS = None


def get_runner(ncores, nloc):
    key = (ncores, nloc)
    if key not in _RUNNER:
        if key not in _CACHE:
            _CACHE[key] = build_program(ncores, nloc)
        _RUNNER[key] = PJRTRunner(_CACHE[key], ncores)
    return _RUNNER[key]


def run(x, nb, pca_w, pca_b, ln_g, ln_b, w_qs, w_ks, mlp_w, mlp_b,
        ncores=NCORES, timing_reps=0):
    global LAST_EXEC_NS
    import time as _time
    n = x.shape[0]
    nloc = n // ncores
    assert nloc * ncores == n
    nbm = np.where(nb < 0, n, nb).astype(np.int64)

    runner = get_runner(ncores, nloc)
    in_maps = _build_inputs(x, nbm, pca_w, pca_b, ln_g, ln_b, w_qs, w_ks,
                            mlp_w, mlp_b, ncores, nloc)
    dev_in = runner.put_inputs(in_maps)
    outs = runner(dev_in)
    if timing_reps:
        times = []
        for _ in range(timing_reps):
            t0 = _time.perf_counter()
            outs = runner(dev_in)
            times.append(_time.perf_counter() - t0)
        LAST_EXEC_NS = int(min(times) * 1e9)
    results = runner.unpack(outs)

    lsm = np.concatenate([r["out_lsm"] for r in results], axis=0)
    meta = np.concatenate([r["out_meta"] for r in results], axis=0)
    attn_parts = np.concatenate([r["out_attn"][:, 0] for r in results], axis=0)
    off_sum = K * K - K
    attn_loss = np.float32(attn_parts.sum() / (off_sum * n))
    return lsm, attn_loss, meta


def kernel(x, nb, pca_w, pca_b, ln_g, ln_b, w_qs, w_ks, mlp_w, mlp_b):
    return run(np.asarray(x, np.float32), np.asarray(nb), np.asarray(pca_w),
               np.asarray(pca_b), np.asarray(ln_g), np.asarray(ln_b),
               np.asarray(w_qs), np.asarray(w_ks), np.asarray(mlp_w),
               np.asarray(mlp_b))
